# revision 7
# baseline (speedup 1.0000x reference)
"""Trainium2 Bass kernel for nn_LossFunction_12532714569881.

Computes, for x: [N=8192, 2, D=256] fp32, w, b scalars:
    P = x[:,0,:]; A = x[:,1,:]
    logits = (P @ A^T) / max(|p_i||a_j|, eps) * w + b        # [N, N]
    loss = -mean_i(log_softmax(logits)[i, i])

Strategy (8 NeuronCores, SPMD, single launch):
  - Row-shard the NxN logits: core c owns rows R=c*1024 .. R+1024.
  - PE does ONLY bf16 matmuls.  All transposes go through the DMA xbar
    (dma_start_transpose, batched [128, t, 128] calls).
  - exp+rowsum of the logits is SPLIT between the scalar engine (ACT
    exp with fused accum, 23 tiles) and the vector engine (9 tiles)
    which uses a Schraudolph bit-trick exp: i16 = round(K16*z + B16)
    written via the int16 convert at the DVE write port, then re-read
    bitcast as bf16 (the int IS the bf16 bit pattern of exp(z)) and
    row-summed with a 4x-mode tensor_scalar reduce.  The diagonal
    (label) term is recomputed exactly in fp32 so the approx error
    lands ~1e-5 on the loss (tolerance 2e-2).
  - Anchor prep is pipelined per 1024-column chunk across the idle
    engines: GPSIMD (no reduce support) does fp32->bf16 casts and the
    normalize; DVE does sum-of-squares (bf16 2x mode) and diag stats;
    ACT only ln/exp chains.
  - Since cos in [-1,1], logits <= |w|+b: the constant shift |w|
    replaces the row-max pass (no overflow); b cancels in softmax.
  - Each core emits one partial scalar = sum of its 1024 row losses;
    the host sums 8 partials and divides by N.

kernel(**inputs) -> np.float32 scalar (shape () like the reference).
"""

import math

import numpy as np

N = 8192
D = 256
NCORES = 8
RPC = N // NCORES          # 1024 rows per core
P = 128                    # partitions
NT_P = RPC // P            # 8 positive tiles / m-chunks
KH = D // P                # 2 k-halves
NB = 512                   # matmul free-dim per instruction
CCOLS = 1024               # columns per prep chunk
NCH = N // CCOLS           # 8 prep chunks
TPC = CCOLS // P           # 8 anchor tiles per chunk
GCOLS = 2048               # columns per exp tile (PSUM tile width)
NGRP = N // GCOLS          # 4 exp groups (= chunk pairs)

# Schraudolph-bf16 exp constants: i16 = round(K16*z + B16) bitcast bf16
K16 = 128.0 / math.log(2.0)
C_SCH = 0.0579             # mean-zero correction (validated offline)
B16 = 16256.0 - C_SCH * 128.0

# which (group, m) exp tiles the DVE takes (rest go to ACT): 9 of 32
DVE_TILE = {(0, 3), (0, 6), (1, 3), (1, 6), (2, 3), (2, 6),
            (3, 2), (3, 5), (3, 7)}

_BUILD_CACHE = {}
_ACT_TABLES_PATCHED = False


def _patch_act_tables():
    """Make both Exp and Ln resolve to the one table set that contains
    them both (natural_log_exp_and_others), so the kernel needs a single
    ACT_TABLE_LOAD instead of thrashing between exp/ln sets."""
    global _ACT_TABLES_PATCHED
    if _ACT_TABLES_PATCHED:
        return
    import concourse.bacc as bacc_mod
    import concourse.bass_interp as interp_mod
    import concourse.mybir as mybir
    from concourse import hw_specs

    AF = mybir.ActivationFunctionType
    orig = hw_specs.get_activation_tables

    def patched(module_arch):
        tabs = orig(module_arch)
        out = {}
        for name, funcs in tabs.items():
            f = set(funcs)
            if name != "natural_log_exp_and_others":
                f.discard(AF.Exp)
                f.discard(AF.Ln)
            out[name] = f
        return out

    bacc_mod.get_activation_tables = patched
    interp_mod.get_activation_tables = patched
    _ACT_TABLES_PATCHED = True


def _build(w: float, b: float):
    from contextlib import ExitStack

    import concourse.bass as bass  # noqa: F401
    import concourse.mybir as mybir
    import concourse.tile as tile
    from concourse import bacc

    _patch_act_tables()

    f32 = mybir.dt.float32
    bf16 = mybir.dt.bfloat16
    i16 = mybir.dt.int16
    AF = mybir.ActivationFunctionType
    ALU = mybir.AluOpType
    AX = mybir.AxisListType

    absw = abs(float(w))
    bias_exp = -absw                      # exp(scale_i*dot - |w|)
    sch_bias = B16 - K16 * absw           # folded into DVE pass-1 scalar2

    nc = bacc.Bacc("TRN2", target_bir_lowering=False, debug=False)

    xp = nc.dram_tensor("xp", [RPC, D], f32, kind="ExternalInput").ap()
    xad = nc.dram_tensor("xad", [RPC, D], f32, kind="ExternalInput").ap()
    xa = nc.dram_tensor("xa", [N, D], f32, kind="ExternalInput").ap()
    out_partial = nc.dram_tensor("partial", [1, 1], f32,
                                 kind="ExternalOutput").ap()

    with tile.TileContext(nc) as tc:
        with ExitStack() as ctx:
            sing = ctx.enter_context(tc.tile_pool(name="sing", bufs=1))
            raw_pool = ctx.enter_context(tc.tile_pool(name="rawp", bufs=8))
            cst_pool = ctx.enter_context(tc.tile_pool(name="cstp", bufs=4))
            sq_pool = ctx.enter_context(tc.tile_pool(name="sqp", bufs=2))
            sqf_pool = ctx.enter_context(tc.tile_pool(name="sqfp", bufs=2))
            act_pool = ctx.enter_context(tc.tile_pool(name="actp", bufs=1))
            i16_pool = ctx.enter_context(tc.tile_pool(name="i16p", bufs=1))
            dmy_pool = ctx.enter_context(tc.tile_pool(name="dmyp", bufs=1))

            # ---- persistent SBUF tensors ------------------------------
            sb_xp = sing.tile([P, NT_P * D], f32, tag="xp")
            sb_xad = sing.tile([P, NT_P * D], f32, tag="xad")
            # normalized anchors / positives, h-plane-major: [P, h, j]
            xa_bf = sing.tile([P, KH * N], bf16, tag="xabf")
            xp_bf = sing.tile([P, KH * RPC], bf16, tag="xpbf")
            ant = [sing.tile([P, N], bf16, tag=f"ant{h}", name=f"ant{h}")
                   for h in range(KH)]
            pnt = [sing.tile([P, RPC], bf16, tag=f"pnt{h}", name=f"pnt{h}")
                   for h in range(KH)]

            ssq_a = sing.tile([P, NCH * TPC], f32, tag="ssqa")
            lns_a = sing.tile([P, NCH * TPC], f32, tag="lnsa")
            inv_a = sing.tile([P, NCH * TPC], f32, tag="inva")
            ssq_pd = sing.tile([P, 2 * NT_P], f32, tag="ssqpd")  # P | XAD
            lns_pd = sing.tile([P, 2 * NT_P], f32, tag="lnspd")
            inv_pd = sing.tile([P, 2 * NT_P], f32, tag="invpd")
            winvp = sing.tile([P, NT_P], f32, tag="winvp")   # w / |p_i|
            s1dve = sing.tile([P, NT_P], f32, tag="s1dve")   # K16*w/|p_i|
            pa = sing.tile([P, NT_P], f32, tag="pa")         # dot(p_i,a_i)
            ssum = sing.tile([P, NT_P * NGRP], f32, tag="ssum")
            srow = sing.tile([P, NT_P], f32, tag="srow")
            lnS = sing.tile([P, NT_P], f32, tag="lnS")
            cosd = sing.tile([P, NT_P], f32, tag="cosd")
            rowloss = sing.tile([P, NT_P], f32, tag="rowloss")
            rsum = sing.tile([P, 1], f32, tag="rsum")
            ones = sing.tile([P, 1], f32, tag="ones")
            bias_t = sing.tile([P, 1], f32, tag="bias_t")
            sc_out = sing.tile([1, 1], f32, tag="sc_out")

            invad = inv_pd[:, NT_P:2 * NT_P]
            inv_p = inv_pd[:, 0:NT_P]

            nc.vector.memset(ones, 1.0)
            nc.vector.memset(bias_t, bias_exp)

            # ---- DMA loads --------------------------------------------
            xa_raw = [None] * NCH

            def load_chunk(eng, ch):
                t = raw_pool.tile([P, TPC * D], f32, tag="xaraw",
                                  name=f"xaraw{ch}")
                xa_raw[ch] = t
                eng.dma_start(
                    out=t.rearrange("p (t d) -> p t d", d=D),
                    in_=xa.rearrange("(c t p) d -> p c t d", p=P, t=TPC)[
                        :, ch, :, :],
                )

            load_chunk(nc.sync, 0)
            nc.scalar.dma_start(
                out=sb_xp.rearrange("p (t d) -> p t d", d=D),
                in_=xp.rearrange("(t p) d -> p t d", p=P),
            )
            load_chunk(nc.sync, 1)
            load_chunk(nc.sync, 2)
            nc.scalar.dma_start(
                out=sb_xad.rearrange("p (t d) -> p t d", d=D),
                in_=xad.rearrange("(t p) d -> p t d", p=P),
            )
            load_chunk(nc.sync, 3)
            load_chunk(nc.sync, 4)
            load_chunk(nc.scalar, 5)
            load_chunk(nc.sync, 6)
            load_chunk(nc.scalar, 7)

            # ---- prep helpers ----------------------------------------
            xa_c = [None] * NCH

            def cast_chunk(ch):
                # GPSIMD: raw fp32 -> bf16 (consumed by DVE sumsq and
                # GP normalize)
                t = cst_pool.tile([P, TPC * D], bf16, tag="xac",
                                  name=f"xac{ch}")
                xa_c[ch] = t
                nc.gpsimd.tensor_scalar(
                    out=t, in0=xa_raw[ch], scalar1=1.0, scalar2=None,
                    op0=ALU.mult,
                )

            def ssq_chunk(ch):
                # DVE: bf16 sum-of-squares per anchor tile (2x mode)
                for t in range(TPC):
                    scr = sq_pool.tile([P, D], bf16, tag="sqscr",
                                       name="sqscr")
                    nc.vector.scalar_tensor_tensor(
                        out=scr,
                        in0=xa_c[ch][:, t * D:(t + 1) * D],
                        scalar=1.0,
                        in1=xa_c[ch][:, t * D:(t + 1) * D],
                        op0=ALU.mult,
                        op1=ALU.mult,
                        accum_out=ssq_a[:, ch * TPC + t:ch * TPC + t + 1],
                    )

            def sumsq_f32(src0, t, acc, col, src1=None):
                scr = sqf_pool.tile([P, D], f32, tag="sqfscr",
                                    name="sqfscr")
                nc.vector.scalar_tensor_tensor(
                    out=scr,
                    in0=src0[:, t * D:(t + 1) * D],
                    scalar=1.0,
                    in1=(src1 if src1 is not None else src0)[
                        :, t * D:(t + 1) * D],
                    op0=ALU.mult,
                    op1=ALU.mult,
                    accum_out=acc[:, col:col + 1],
                )

            def inv_chain(ch):
                lo, hi = ch * TPC, (ch + 1) * TPC
                nc.scalar.activation(lns_a[:, lo:hi], ssq_a[:, lo:hi],
                                     AF.Ln)
                nc.scalar.activation(inv_a[:, lo:hi], lns_a[:, lo:hi],
                                     AF.Exp, scale=-0.5)

            def norm_chunk(ch):
                # GPSIMD: fused normalize (bf16 in/out), h-plane-split
                for t in range(TPC):
                    gt = ch * TPC + t
                    nc.gpsimd.tensor_scalar(
                        out=xa_bf.rearrange("p (h j) -> p h j", h=KH)[
                            :, :, gt * P:(gt + 1) * P],
                        in0=xa_c[ch].rearrange(
                            "p (t h dk) -> p t h dk", h=KH, dk=P)[:, t],
                        scalar1=inv_a[:, gt:gt + 1],
                        scalar2=None,
                        op0=ALU.mult,
                    )

            def xbar_chunk(ch):
                for h in range(KH):
                    nc.sync.dma_start_transpose(
                        out=ant[h].rearrange("p (c t f) -> p (c t) f",
                                             c=NCH, f=P)[
                            :, ch * TPC:(ch + 1) * TPC, :],
                        in_=xa_bf[:, h * N + ch * CCOLS:
                                  h * N + (ch + 1) * CCOLS],
                    )

            # ---- exp consumers ---------------------------------------
            def exp_act(ps, g, m):
                scr = act_pool.tile([P, GCOLS], bf16, tag="actscr",
                                    name="actscr")
                nc.scalar.activation(
                    scr, ps, AF.Exp,
                    bias=bias_t[:, 0:1],
                    scale=winvp[:, m:m + 1],
                    accum_out=ssum[:, m * NGRP + g: m * NGRP + g + 1],
                )

            def exp_dve(ps, g, m):
                scr_i = i16_pool.tile([P, GCOLS], i16, tag="i16scr",
                                      name="i16scr")
                nc.vector.tensor_scalar(
                    out=scr_i,
                    in0=ps,
                    scalar1=s1dve[:, m:m + 1],
                    scalar2=float(sch_bias),
                    op0=ALU.mult,
                    op1=ALU.add,
                )
                dmy = dmy_pool.tile([P, GCOLS], bf16, tag="dmyscr",
                                    name="dmyscr")
                nc.vector.tensor_scalar(
                    out=dmy,
                    in0=scr_i[:].bitcast(bf16),
                    scalar1=1.0,
                    scalar2=None,
                    op0=ALU.mult,
                    op1=ALU.add,    # reduce op for accum_out
                    accum_out=ssum[:, m * NGRP + g: m * NGRP + g + 1],
                )

            # ---- prep phase -------------------------------------------
            # GP: cast0, xpcast, cast1, cast2  (rest interleaved below)
            cast_chunk(0)
            for h in range(KH):
                nc.gpsimd.tensor_copy(
                    xp_bf.rearrange("p (h m dk) -> p h m dk",
                                    h=KH, dk=P)[:, h],
                    sb_xp.rearrange("p (m h dk) -> p h m dk",
                                    h=KH, dk=P)[:, h],
                )
            cast_chunk(1)
            cast_chunk(2)

            # DVE: ssq0, xp-ssq, scales, ssq1-3
            ssq_chunk(0)
            inv_chain(0)                                  # ACT
            for t in range(NT_P):
                sumsq_f32(sb_xp, t, ssq_pd, t)
            nc.scalar.activation(lns_pd[:, 0:NT_P], ssq_pd[:, 0:NT_P],
                                 AF.Ln)
            nc.scalar.activation(inv_pd[:, 0:NT_P], lns_pd[:, 0:NT_P],
                                 AF.Exp, scale=-0.5)
            nc.vector.tensor_scalar_mul(winvp, inv_p, float(w))
            nc.vector.tensor_scalar_mul(s1dve, inv_p, float(w) * K16)

            # GP: norm0 then alternate cast/norm
            norm_chunk(0)
            cast_chunk(3)

            ssq_chunk(1)                                  # DVE
            inv_chain(1)                                  # ACT
            norm_chunk(1)                                 # GP
            cast_chunk(4)
            ssq_chunk(2)
            inv_chain(2)
            norm_chunk(2)
            cast_chunk(5)
            ssq_chunk(3)
            inv_chain(3)
            norm_chunk(3)
            cast_chunk(6)

            # sync ring: pnt transposes, then chunk xbars as ready
            for h in range(KH):
                nc.sync.dma_start_transpose(
                    out=pnt[h].rearrange("p (t f) -> p t f", f=P),
                    in_=xp_bf[:, h * RPC:(h + 1) * RPC],
                )
            xbar_chunk(0)
            xbar_chunk(1)
            xbar_chunk(2)
            xbar_chunk(3)

            # ---- main loop --------------------------------------------
            with tc.tile_pool(name="psM", bufs=2, space="PSUM") as psM:
                for g in range(NGRP):
                    # drip remaining prep one group ahead
                    if g == 0:
                        ssq_chunk(4)                      # DVE
                        inv_chain(4)                      # ACT
                        norm_chunk(4)                     # GP
                        cast_chunk(7)                     # GP
                        ssq_chunk(5)
                        inv_chain(5)
                        norm_chunk(5)
                        xbar_chunk(4)
                        xbar_chunk(5)
                    elif g == 1:
                        ssq_chunk(6)
                        inv_chain(6)
                        norm_chunk(6)
                        ssq_chunk(7)
                        inv_chain(7)
                        norm_chunk(7)
                        xbar_chunk(6)
                        xbar_chunk(7)
                    elif g == 2:
                        # diag stats (tail-only; fp32-exact)
                        for t in range(NT_P):
                            sumsq_f32(sb_xad, t, ssq_pd, NT_P + t)
                        for t in range(NT_P):
                            sumsq_f32(sb_xp, t, pa, t, src1=sb_xad)
                        nc.scalar.activation(lns_pd[:, NT_P:],
                                             ssq_pd[:, NT_P:], AF.Ln)
                        nc.scalar.activation(inv_pd[:, NT_P:],
                                             lns_pd[:, NT_P:],
                                             AF.Exp, scale=-0.5)
                    for m in range(NT_P):
                        ps = psM.tile([P, GCOLS], f32, tag="psmm",
                                      name="psmm")
                        for h in range(KH):
                            for nn in range(GCOLS // NB):
                                nc.tensor.matmul(
                                    ps[:, nn * NB:(nn + 1) * NB],
                                    pnt[h][:, m * P:(m + 1) * P],
                                    ant[h][:, g * GCOLS + nn * NB:
                                           g * GCOLS + (nn + 1) * NB],
                                    start=(h == 0),
                                    stop=(h == KH - 1),
                                )
                        if (g, m) in DVE_TILE:
                            exp_dve(ps, g, m)
                        else:
                            exp_act(ps, g, m)

            # ---- tail -------------------------------------------------
            nc.vector.tensor_reduce(
                srow,
                ssum.rearrange("p (m g) -> p m g", g=NGRP),
                axis=AX.X,
                op=ALU.add,
            )
            nc.scalar.activation(lnS, srow, AF.Ln)
            # rowloss = lnS + |w| - w*pa*inv_p*invad
            nc.vector.tensor_mul(cosd, pa, inv_p)
            nc.vector.tensor_mul(cosd, cosd, invad)
            nc.vector.scalar_tensor_tensor(
                out=rowloss,
                in0=cosd,
                scalar=-float(w),
                in1=lnS,
                op0=ALU.mult,
                op1=ALU.add,
            )
            nc.vector.tensor_scalar_add(rowloss, rowloss, absw)
            nc.vector.reduce_sum(rsum, rowloss, axis=AX.X)

            with tc.tile_pool(name="psF", bufs=1, space="PSUM") as psF:
                pfin = psF.tile([1, 1], f32, tag="pfin")
                nc.tensor.matmul(pfin, rsum, ones, start=True, stop=True)
                nc.vector.tensor_copy(sc_out, pfin)
            nc.sync.dma_start(out=out_partial, in_=sc_out)

    nc.compile()
    return nc


def _get_nc(w: float, b: float):
    key = (float(w), float(b))
    if key not in _BUILD_CACHE:
        _BUILD_CACHE[key] = _build(float(w), float(b))
    return _BUILD_CACHE[key]


def kernel(x, w, b, epoch=None, **_unused):
    from concourse.bass_utils import run_bass_kernel_spmd

    x = np.asarray(x, dtype=np.float32)
    w_f = float(np.asarray(w))
    b_f = float(np.asarray(b))
    assert x.shape == (N, 2, D), x.shape

    nc = _get_nc(w_f, b_f)

    xa_full = np.ascontiguousarray(x[:, 1, :])
    in_maps = []
    for c in range(NCORES):
        r0 = c * RPC
        in_maps.append({
            "xp": np.ascontiguousarray(x[r0:r0 + RPC, 0, :]),
            "xad": np.ascontiguousarray(x[r0:r0 + RPC, 1, :]),
            "xa": xa_full,
        })

    res = run_bass_kernel_spmd(nc, in_maps, list(range(NCORES)))
    total = 0.0
    for c in range(NCORES):
        total += float(res.results[c]["partial"][0, 0])
    loss = total / N
    return np.float32(loss)


# revision 13
# speedup vs baseline: 4.0915x; 4.0915x over previous
"""Trainium2 Bass kernel for nn_LossFunction_12532714569881.

Computes, for x: [N=8192, 2, D=256] fp32, w, b scalars:
    P = x[:,0,:]; A = x[:,1,:]
    logits = (P @ A^T) / max(|p_i||a_j|, eps) * w + b        # [N, N]
    loss = -mean_i(log_softmax(logits)[i, i])

Strategy (8 NeuronCores, SPMD, single launch):
  - Row-shard the NxN logits: core c owns rows R=c*1024 .. R+1024.
    The host ROTATES each core's anchor matrix by -R so the diagonal
    (label) entries land in local column chunk 0 uniformly across
    cores; the diagonal dot is then extracted from the group-0 PSUM
    tiles with a masked scalar_tensor_tensor (identity-mask multiply,
    fused accumulate) instead of a separate fp32 dot pass.
  - PE does ONLY bf16 matmuls; all transposes go through the DMA xbar
    (dma_start_transpose, batched [128, t, 128] calls).
  - exp+rowsum of the logits is SPLIT: scalar engine (ACT exp, fused
    accum, 29 tiles) + vector engine (3 tiles) via a Schraudolph
    bit-trick exp (affine -> int16 convert at the DVE write port;
    re-read bitcast as bf16 = exp(z); row-sum with tensor_scalar
    reduce).  Approx error lands ~1e-5 on the loss (tolerance 2e-2).
  - Anchor prep (sum-of-squares, ln/exp inverse-norm chain, fused
    normalize+bf16 cast) is chunk-pipelined: DVE does sumsq+normalize,
    ACT only the ln/exp chains.  GPSIMD proved ~15x too slow for bulk
    elementwise work and is demoted to mask building.
  - Since cos in [-1,1], logits <= |w|+b: constant shift |w| replaces
    the row-max pass; b cancels in softmax.
  - Each core emits one partial scalar; host sums 8 partials / N.

kernel(**inputs) -> np.float32 scalar (shape () like the reference).
"""

import math

import numpy as np

N = 8192
D = 256
NCORES = 8
RPC = N // NCORES          # 1024 rows per core
P = 128                    # partitions
NT_P = RPC // P            # 8 positive tiles / m-chunks
KH = D // P                # 2 k-halves
NB = 512                   # matmul free-dim per instruction
CCOLS = 1024               # columns per prep chunk
NCH = N // CCOLS           # 8 prep chunks
TPC = CCOLS // P           # 8 anchor tiles per chunk
GCOLS = 2048               # columns per exp tile (PSUM tile width)
NGRP = N // GCOLS          # 4 exp groups (= chunk pairs)

# Schraudolph-bf16 exp constants: i16 = round(K16*z + B16) bitcast bf16
K16 = 128.0 / math.log(2.0)
C_SCH = 0.0579             # mean-zero correction (validated offline)
B16 = 16256.0 - C_SCH * 128.0

# which (group, m) exp tiles the DVE takes (rest go to ACT)
DVE_TILE = {(1, 3), (2, 4), (3, 3)}

LDW_OPT = False
_BUILD_CACHE = {}
_ACT_TABLES_PATCHED = False
_LDW_OPT_PATCHED = False


def _patch_ldw_opt():
    """walrus's redundant-LDWEIGHTS elision is hardcoded off in
    bass_utils; consecutive same-weight matmuls then re-load the PE
    array every instruction.  Rewrite the flag on the walrus command
    line.  Validated against the reference output."""
    global _LDW_OPT_PATCHED
    if _LDW_OPT_PATCHED or not LDW_OPT:
        return
    import concourse.bass_utils as bu

    orig_run = bu.run_command

    def patched(argv, **kwargs):
        argv = [a.replace("--enable-ldw-opt=false", "--enable-ldw-opt=true")
                if isinstance(a, str) else a for a in argv]
        return orig_run(argv, **kwargs)

    bu.run_command = patched
    _LDW_OPT_PATCHED = True


def _patch_act_tables():
    """Make both Exp and Ln resolve to the one table set that contains
    them both (natural_log_exp_and_others): a single ACT_TABLE_LOAD."""
    global _ACT_TABLES_PATCHED
    if _ACT_TABLES_PATCHED:
        return
    import concourse.bacc as bacc_mod
    import concourse.bass_interp as interp_mod
    import concourse.mybir as mybir
    from concourse import hw_specs

    AF = mybir.ActivationFunctionType
    orig = hw_specs.get_activation_tables

    def patched(module_arch):
        tabs = orig(module_arch)
        out = {}
        for name, funcs in tabs.items():
            f = set(funcs)
            if name != "natural_log_exp_and_others":
                f.discard(AF.Exp)
                f.discard(AF.Ln)
            out[name] = f
        return out

    bacc_mod.get_activation_tables = patched
    interp_mod.get_activation_tables = patched
    _ACT_TABLES_PATCHED = True


def _build(w: float, b: float):
    from contextlib import ExitStack

    import concourse.bass as bass  # noqa: F401
    import concourse.mybir as mybir
    import concourse.tile as tile
    from concourse import bacc
    from concourse.masks import make_identity

    _patch_act_tables()
    _patch_ldw_opt()

    f32 = mybir.dt.float32
    bf16 = mybir.dt.bfloat16
    i16 = mybir.dt.int16
    AF = mybir.ActivationFunctionType
    ALU = mybir.AluOpType
    AX = mybir.AxisListType

    absw = abs(float(w))
    bias_exp = -absw                      # exp(scale_i*dot - |w|)
    sch_bias = B16 - K16 * absw           # folded into DVE pass-1 scalar2

    nc = bacc.Bacc("TRN2", target_bir_lowering=False, debug=False)

    xp = nc.dram_tensor("xp", [RPC, D], f32, kind="ExternalInput").ap()
    xa = nc.dram_tensor("xa", [N, D], f32, kind="ExternalInput").ap()
    out_partial = nc.dram_tensor("partial", [1, 1], f32,
                                 kind="ExternalOutput").ap()

    with tile.TileContext(nc) as tc:
        with ExitStack() as ctx:
            sing = ctx.enter_context(tc.tile_pool(name="sing", bufs=1))
            raw_pool = ctx.enter_context(tc.tile_pool(name="rawp", bufs=8))
            sq_pool = ctx.enter_context(tc.tile_pool(name="sqp", bufs=2))
            dg_pool = ctx.enter_context(tc.tile_pool(name="dgp", bufs=2))
            act_pool = ctx.enter_context(tc.tile_pool(name="actp", bufs=1))
            i16_pool = ctx.enter_context(tc.tile_pool(name="i16p", bufs=1))
            dmy_pool = ctx.enter_context(tc.tile_pool(name="dmyp", bufs=1))

            # ---- persistent SBUF tensors ------------------------------
            sb_xp = sing.tile([P, NT_P * D], f32, tag="xp")
            # normalized anchors / raw positives, h-plane-major: [P,h,j]
            xa_bf = sing.tile([P, KH * N], bf16, tag="xabf")
            xp_bf = sing.tile([P, KH * RPC], bf16, tag="xpbf")
            ant = [sing.tile([P, N], bf16, tag=f"ant{h}", name=f"ant{h}")
                   for h in range(KH)]
            pnt = [sing.tile([P, RPC], bf16, tag=f"pnt{h}", name=f"pnt{h}")
                   for h in range(KH)]

            ssq_a = sing.tile([P, NCH * TPC], f32, tag="ssqa")
            lns_a = sing.tile([P, NCH * TPC], f32, tag="lnsa")
            inv_a = sing.tile([P, NCH * TPC], f32, tag="inva")
            ssq_p = sing.tile([P, NT_P], f32, tag="ssqp")
            lns_p = sing.tile([P, NT_P], f32, tag="lnsp")
            inv_p = sing.tile([P, NT_P], f32, tag="invp")
            winvp = sing.tile([P, NT_P], f32, tag="winvp")   # w / |p_i|
            s1dve = sing.tile([P, NT_P], f32, tag="s1dve")   # K16*w/|p_i|
            dotd = sing.tile([P, NT_P], f32, tag="dotd")     # p_i . a_i
            ssum = sing.tile([P, NT_P * NGRP], f32, tag="ssum")
            srow = sing.tile([P, NT_P], f32, tag="srow")
            lnS = sing.tile([P, NT_P], f32, tag="lnS")
            roww = sing.tile([P, NT_P], f32, tag="roww")
            rowloss = sing.tile([P, NT_P], f32, tag="rowloss")
            rsum = sing.tile([P, 1], f32, tag="rsum")
            ones = sing.tile([P, 1], f32, tag="ones")
            bias_t = sing.tile([P, 1], f32, tag="bias_t")
            ident = sing.tile([P, P], f32, tag="ident")
            sc_out = sing.tile([1, 1], f32, tag="sc_out")

            nc.vector.memset(ones, 1.0)
            nc.vector.memset(bias_t, bias_exp)
            make_identity(nc, ident[:])          # gpsimd; idle engine

            # ---- DMA loads (sync HWDGE ring; no xad input) ------------
            xa_raw = [None] * NCH

            def load_chunk(ch):
                t = raw_pool.tile([P, TPC * D], f32, tag="xaraw",
                                  name=f"xaraw{ch}")
                xa_raw[ch] = t
                nc.sync.dma_start(
                    out=t.rearrange("p (t d) -> p t d", d=D),
                    in_=xa.rearrange("(c t p) d -> p c t d", p=P, t=TPC)[
                        :, ch, :, :],
                )

            load_chunk(0)
            nc.scalar.dma_start(
                out=sb_xp.rearrange("p (t d) -> p t d", d=D),
                in_=xp.rearrange("(t p) d -> p t d", p=P),
            )
            for ch in range(1, NCH):
                load_chunk(ch)

            # ---- prep helpers ----------------------------------------
            def ssq_chunk(ch):
                for t in range(TPC):
                    scr = sq_pool.tile([P, D], bf16, tag="sqscr",
                                       name="sqscr")
                    nc.vector.scalar_tensor_tensor(
                        out=scr,
                        in0=xa_raw[ch][:, t * D:(t + 1) * D],
                        scalar=1.0,
                        in1=xa_raw[ch][:, t * D:(t + 1) * D],
                        op0=ALU.mult,
                        op1=ALU.mult,
                        accum_out=ssq_a[:, ch * TPC + t:ch * TPC + t + 1],
                    )

            def inv_chain(ch):
                lo, hi = ch * TPC, (ch + 1) * TPC
                nc.scalar.activation(lns_a[:, lo:hi], ssq_a[:, lo:hi],
                                     AF.Ln)
                nc.scalar.activation(inv_a[:, lo:hi], lns_a[:, lo:hi],
                                     AF.Exp, scale=-0.5)

            def norm_chunk(ch):
                # DVE: fused normalize + bf16 cast, h-plane-split output
                for t in range(TPC):
                    gt = ch * TPC + t
                    nc.vector.tensor_scalar(
                        out=xa_bf.rearrange("p (h j) -> p h j", h=KH)[
                            :, :, gt * P:(gt + 1) * P],
                        in0=xa_raw[ch].rearrange(
                            "p (t h dk) -> p t h dk", h=KH, dk=P)[:, t],
                        scalar1=inv_a[:, gt:gt + 1],
                        scalar2=None,
                        op0=ALU.mult,
                    )

            def xbar_chunk(ch):
                for h in range(KH):
                    nc.sync.dma_start_transpose(
                        out=ant[h].rearrange("p (c t f) -> p (c t) f",
                                             c=NCH, f=P)[
                            :, ch * TPC:(ch + 1) * TPC, :],
                        in_=xa_bf[:, h * N + ch * CCOLS:
                                  h * N + (ch + 1) * CCOLS],
                    )

            # ---- exp consumers + diag extract ------------------------
            def exp_act(ps, g, m):
                scr = act_pool.tile([P, GCOLS], bf16, tag="actscr",
                                    name="actscr")
                nc.scalar.activation(
                    scr, ps, AF.Exp,
                    bias=bias_t[:, 0:1],
                    scale=winvp[:, m:m + 1],
                    accum_out=ssum[:, m * NGRP + g: m * NGRP + g + 1],
                )

            def exp_dve(ps, g, m):
                scr_i = i16_pool.tile([P, GCOLS], i16, tag="i16scr",
                                      name="i16scr")
                nc.vector.tensor_scalar(
                    out=scr_i,
                    in0=ps,
                    scalar1=s1dve[:, m:m + 1],
                    scalar2=float(sch_bias),
                    op0=ALU.mult,
                    op1=ALU.add,
                )
                dmy = dmy_pool.tile([P, GCOLS], bf16, tag="dmyscr",
                                    name="dmyscr")
                nc.vector.tensor_scalar(
                    out=dmy,
                    in0=scr_i[:].bitcast(bf16),
                    scalar1=1.0,
                    scalar2=None,
                    op0=ALU.mult,
                    op1=ALU.add,    # reduce op for accum_out
                    accum_out=ssum[:, m * NGRP + g: m * NGRP + g + 1],
                )

            def diag_extract(ps, m):
                # dotd[:, m] = sum_j ps[p, m*128+j] * I[p, j]  (=diag)
                scr = dg_pool.tile([P, P], bf16, tag="dgscr",
                                   name="dgscr")
                nc.vector.scalar_tensor_tensor(
                    out=scr,
                    in0=ps[:, (m % (GCOLS // P)) * P:
                           (m % (GCOLS // P)) * P + P],
                    scalar=1.0,
                    in1=ident,
                    op0=ALU.mult,
                    op1=ALU.mult,
                    accum_out=dotd[:, m:m + 1],
                )

            # ---- prep phase (emission order == engine program order) --
            # DVE: xpcast, ssq0, norm0, xp-ssq, scales, ssq/norm 1-3
            for h in range(KH):
                nc.vector.tensor_copy(
                    xp_bf.rearrange("p (h m dk) -> p h m dk",
                                    h=KH, dk=P)[:, h],
                    sb_xp.rearrange("p (m h dk) -> p h m dk",
                                    h=KH, dk=P)[:, h],
                )
            ssq_chunk(0)
            inv_chain(0)                                  # ACT
            norm_chunk(0)
            for t in range(NT_P):
                scr = sq_pool.tile([P, D], bf16, tag="sqscr", name="sqscr")
                nc.vector.scalar_tensor_tensor(
                    out=scr,
                    in0=sb_xp[:, t * D:(t + 1) * D],
                    scalar=1.0,
                    in1=sb_xp[:, t * D:(t + 1) * D],
                    op0=ALU.mult,
                    op1=ALU.mult,
                    accum_out=ssq_p[:, t:t + 1],
                )
            nc.scalar.activation(lns_p, ssq_p, AF.Ln)
            nc.scalar.activation(inv_p, lns_p, AF.Exp, scale=-0.5)
            nc.vector.tensor_scalar_mul(winvp, inv_p, float(w))
            nc.vector.tensor_scalar_mul(s1dve, inv_p, float(w) * K16)
            for ch in (1, 2, 3):
                ssq_chunk(ch)
                inv_chain(ch)
                norm_chunk(ch)

            # sync ring: pnt transposes, then chunk xbars in order
            for h in range(KH):
                nc.sync.dma_start_transpose(
                    out=pnt[h].rearrange("p (t f) -> p t f", f=P),
                    in_=xp_bf[:, h * RPC:(h + 1) * RPC],
                )
            for ch in range(4):
                xbar_chunk(ch)

            # ---- main loop --------------------------------------------
            with tc.tile_pool(name="psM", bufs=2, space="PSUM") as psM:
                for g in range(NGRP):
                    # drip remaining prep one group ahead (xbar AFTER its
                    # norm: emission order is dependency order for Tile)
                    if g == 1:
                        for ch in (4, 5):
                            ssq_chunk(ch)
                            inv_chain(ch)
                            norm_chunk(ch)
                            xbar_chunk(ch)
                    elif g == 2:
                        for ch in (6, 7):
                            ssq_chunk(ch)
                            inv_chain(ch)
                            norm_chunk(ch)
                            xbar_chunk(ch)
                    for m in range(NT_P):
                        ps = psM.tile([P, GCOLS], f32, tag="psmm",
                                      name="psmm")
                        for h in range(KH):
                            for nn in range(GCOLS // NB):
                                nc.tensor.matmul(
                                    ps[:, nn * NB:(nn + 1) * NB],
                                    pnt[h][:, m * P:(m + 1) * P],
                                    ant[h][:, g * GCOLS + nn * NB:
                                           g * GCOLS + (nn + 1) * NB],
                                    start=(h == 0),
                                    stop=(h == KH - 1),
                                )
                        # group 0 holds the (rotated) diagonal blocks:
                        # tile (0, m) has diag at columns m*128..m*128+128
                        # for m<2 -> tile g0 covers cols 0..2047 = m 0..15?
                        # GCOLS=2048: group 0 = local cols 0..2047, which
                        # contains diag blocks for m in 0..15 -> all m of
                        # rows 0..1023? rows are m*128..: diag col for
                        # row r=m*128+p is m*128+p, inside group g =
                        # (m*128)//2048 = m//16 -> group 0 for m<16: all
                        # 8 m-tiles. Extract in tile (g, m) iff
                        # g == m // (GCOLS // P) ... see below.
                        if g == (m * P) // GCOLS:
                            diag_extract(ps, m)
                        if (g, m) in DVE_TILE:
                            exp_dve(ps, g, m)
                        else:
                            exp_act(ps, g, m)

            # ---- tail -------------------------------------------------
            nc.vector.tensor_reduce(
                srow,
                ssum.rearrange("p (m g) -> p m g", g=NGRP),
                axis=AX.X,
                op=ALU.add,
            )
            nc.scalar.activation(lnS, srow, AF.Ln)
            # rowloss = lnS + |w| - winvp*dotd
            nc.vector.tensor_mul(roww, dotd, winvp)
            nc.vector.scalar_tensor_tensor(
                out=rowloss,
                in0=roww,
                scalar=-1.0,
                in1=lnS,
                op0=ALU.mult,
                op1=ALU.add,
            )
            nc.vector.tensor_scalar_add(rowloss, rowloss, absw)
            nc.vector.reduce_sum(rsum, rowloss, axis=AX.X)

            with tc.tile_pool(name="psF", bufs=1, space="PSUM") as psF:
                pfin = psF.tile([1, 1], f32, tag="pfin")
                nc.tensor.matmul(pfin, rsum, ones, start=True, stop=True)
                nc.vector.tensor_copy(sc_out, pfin)
            nc.sync.dma_start(out=out_partial, in_=sc_out)

    nc.compile()
    return nc


def _get_nc(w: float, b: float):
    key = (float(w), float(b))
    if key not in _BUILD_CACHE:
        _BUILD_CACHE[key] = _build(float(w), float(b))
    return _BUILD_CACHE[key]


def build_in_maps(x):
    xa_full = np.ascontiguousarray(x[:, 1, :])
    in_maps = []
    for c in range(NCORES):
        r0 = c * RPC
        in_maps.append({
            "xp": np.ascontiguousarray(x[r0:r0 + RPC, 0, :]),
            # rotate so this core's diagonal block is local chunk 0
            "xa": np.ascontiguousarray(np.roll(xa_full, -r0, axis=0)),
        })
    return in_maps


def kernel(x, w, b, epoch=None, **_unused):
    from concourse.bass_utils import run_bass_kernel_spmd

    x = np.asarray(x, dtype=np.float32)
    w_f = float(np.asarray(w))
    b_f = float(np.asarray(b))
    assert x.shape == (N, 2, D), x.shape

    nc = _get_nc(w_f, b_f)

    in_maps = build_in_maps(x)

    res = run_bass_kernel_spmd(nc, in_maps, list(range(NCORES)))
    total = 0.0
    for c in range(NCORES):
        total += float(res.results[c]["partial"][0, 0])
    loss = total / N
    return np.float32(loss)
